# revision 11
# baseline (speedup 1.0000x reference)
"""Differentiable top-k (Sinkhorn) Trainium2 kernel.

Math: the reference runs 100 log-domain Sinkhorn iterations on
log_P0[i,j] = -(s_i - sorted_j)^2/eps, then sums exp(log_P) over the
first K=50 columns.

This kernel exploits three structural facts:

1. Conjugating by the sort permutation, the whole iteration runs in the
   RAW index domain on the symmetric kernel matrix
   K[i,j] = exp(-(s_i - s_j)^2/eps); no on-chip sort is needed.  The
   only sorted-domain quantity required is the top-50 mask
   m[j] = [rank_j < 50], obtained by comparison counting.

2. The alternating column/row normalizations collapse to the scalar
   chain w <- 1/(K w); its linearization has spectrum {1} U [0, ~0.977],
   so the DAMPED iteration  w <- 0.5 w + 0.5/(K w)  contracts every
   error mode at <= ~0.5 per step instead of 0.977: three damped rounds
   land within rv ~ 4e-6 of the T=100 reference output (the T=100
   truncation transient itself is only ~6e-3 absmax from the fixed
   point, far inside the 2e-2 gate).  Storing W = sqrt(2) w makes the
   update exactly  W <- 0.5 W + reciprocal(K W)  (one vector reciprocal
   + one scalar_tensor_tensor); the sqrt(2) gauge cancels in the output
   because out = v * (K (m*u)) is invariant under u -> c u, v -> v/c.

3. Output: u = W_3, v = reciprocal(K W_3),
   out = v * (K (m*W_3)), all elementwise in raw order.

Latency engineering (the kernel is dependency-bound, not
throughput-bound): a dummy activation hoists the ~2.7us ACT table load
to t=0; the score row-replica s_rep is built by a PE broadcast matmul
(ones^T @ s_row) straight into PSUM (a 256KB HBM DMA takes ~5.6us, the
matmul 1.5us); score-layout transposes run on the PE; both batches'
outputs are PE-transposed into one [4, 256] tile and leave through a
single contiguous DMA; input DMAs are spread across the three DMA
queues (sync/gpsimd/scalar).
"""

import numpy as np

import concourse.bacc as bacc
import concourse.mybir as mybir
from concourse import tile
from concourse.bass_utils import run_bass_kernel_spmd

F32 = mybir.dt.float32
F16 = mybir.dt.float16
BF16 = mybir.dt.bfloat16

B_FULL = 16
N = 512
NB = N // 128  # 4 row/col blocks
TK = 50
N_CORES = 8
B_LOC = B_FULL // N_CORES  # batches per core
ROUNDS = 3  # damped w-update rounds (then +1 matvec for v, +1 for out)
SQ2 = float(np.sqrt(2.0))
RT1000 = float(np.sqrt(1000.0))  # sqrt(1/eps)


def _build():
    nc = bacc.Bacc("TRN2", target_bir_lowering=False, debug=False)

    scores_d = nc.declare_dram_parameter("scores", [B_LOC, N], F32, isOutput=False)
    eye128_d = nc.declare_dram_parameter("eye128", [128, 128], F32, isOutput=False)
    # out_flat[p, b*128 + c] = out_full[b, c*128 + p]
    out_d = nc.declare_dram_parameter("out", [4, B_LOC * 128], F32, isOutput=True)

    with nc.allow_low_precision(reason="fp16 sinkhorn iterates"), \
         tile.TileContext(nc) as tc:
        with tc.tile_pool(name="sb", bufs=1) as sb, \
             tc.tile_pool(name="scr", bufs=2) as scr, \
             tc.tile_pool(name="wp", bufs=2) as wp, \
             tc.tile_pool(name="ps_w", bufs=1, space="PSUM") as ps_w, \
             tc.tile_pool(name="ps_r", bufs=1, space="PSUM") as ps_r, \
             tc.tile_pool(name="ps_t", bufs=1, space="PSUM") as ps_t:

            # ---- table-load hoist: dummy activation touching only zero_col
            zero_col = sb.tile([128, 1], F32, name="zero_col", tag="zero_col")
            nc.vector.memset(zero_col[:], 0.0)
            dummy = sb.tile([128, 1], F32, name="dummy", tag="dummy")
            nc.scalar.activation(
                dummy[:], zero_col[:], mybir.ActivationFunctionType.Square,
                bias=0.0, scale=1.0,
            )
            ones_row = sb.tile([1, 128], F32, name="ones_row", tag="ones_row")
            nc.vector.memset(ones_row[:], 1.0)

            # ---- input DMAs, spread across the three DMA queues ----
            s_rows, rows4 = {}, {}
            for b in range(B_LOC):
                s_rows[b] = sb.tile([1, N], F32, name=f"s_row{b}", tag=f"s_row{b}")
                rows4[b] = sb.tile([4, 128], F32, name=f"rows4_{b}", tag=f"rows4_{b}")
            eye4 = sb.tile([4, 4], F32, name="eye4", tag="eye4")
            ones4 = sb.tile([4, 4], F32, name="ones4", tag="ones4")
            eye128 = sb.tile([128, 128], F32, name="eye128", tag="eye128")
            # eye4 built on-chip (gpsimd): no DMA on the scalar queue, and
            # the PE transposes are not gated on a late identity load
            nc.gpsimd.memset(ones4[:], 1.0)
            nc.gpsimd.affine_select(
                eye4[:], ones4[:], [[1, 4]], mybir.AluOpType.is_equal, 0.0,
                base=0, channel_multiplier=-1,
            )
            nc.sync.dma_start(
                s_rows[0][:], scores_d[0].rearrange("(o n) -> o n", o=1)
            )
            nc.gpsimd.dma_start(
                s_rows[1][:], scores_d[1].rearrange("(o n) -> o n", o=1)
            )
            nc.sync.dma_start(rows4[0][:], scores_d[0].rearrange("(p c) -> p c", p=4))
            nc.gpsimd.dma_start(rows4[1][:], scores_d[1].rearrange("(p c) -> p c", p=4))
            nc.gpsimd.dma_start(eye128[:], eye128_d[:])

            # ---- s_rep[b] = ones^T @ s_row (PSUM [128, N] broadcast), and
            #      s_part[p, c] = s[c*128+p] via PE transpose.  Separate
            #      PSUM tiles per batch: tile-granular dependency tracking
            #      must not make batch 0's consumers wait on batch 1. ----
            s_reps, s_parts, nsqs = {}, {}, {}
            for b in range(B_LOC):
                s_reps[b] = ps_r.tile([128, N], F32, name=f"srep{b}", tag=f"srep{b}")
                nc.tensor.matmul(
                    s_reps[b][:], ones_row[:], s_rows[b][:], start=True, stop=True
                )
                pt = ps_t.tile([128, 4], F32, name=f"pst{b}", tag=f"pst{b}")
                nc.tensor.transpose(pt[:], rows4[b][:], eye4[:])
                s_parts[b] = sb.tile([128, NB], F32, name=f"s_part{b}", tag=f"s_part{b}")
                nc.vector.tensor_copy(s_parts[b][:], pt[:])
                nsqs[b] = sb.tile([128, NB], F32, name=f"nsq{b}", tag=f"nsq{b}")
                nc.vector.tensor_scalar(
                    out=nsqs[b][:], in0=s_parts[b][:], scalar1=-RT1000,
                    scalar2=None, op0=mybir.AluOpType.mult,
                )

            # ---- K tiles on scalar: 4 biased Squares + one fused Exp ----
            # kt[b][q, jo*N + i] = exp(-1000 (s_{jo*128+q} - s_i)^2)
            kts = {}
            for b in range(B_LOC):
                sq_all = scr.tile([128, NB * N], F32, name="sq_all", tag="sq_all", bufs=1)
                for jo in range(NB):
                    nc.scalar.activation(
                        sq_all[:, jo * N : (jo + 1) * N], s_reps[b][:],
                        mybir.ActivationFunctionType.Square,
                        bias=nsqs[b][:, jo : jo + 1], scale=RT1000,
                    )
                kts[b] = sb.tile([128, NB * N], F16, name=f"kt{b}", tag=f"kt{b}")
                nc.scalar.activation(
                    kts[b][:], sq_all[:], mybir.ActivationFunctionType.Exp,
                    bias=zero_col[:], scale=-1.0,
                )

            # ---- ranks on vector: rank[j] = #{i: s_i > s_j}, then mask ----
            rank_parts, masks = {}, {}
            for b in range(B_LOC):
                rank_parts[b] = sb.tile([128, NB], F32, name=f"rank{b}", tag=f"rank{b}")
            for c in range(NB):
                for b in range(B_LOC):
                    cm = scr.tile(
                        [128, N], BF16, name=f"cmp{b}_{c}", tag=f"cmp{b}_{c % 2}"
                    )
                    nc.vector.tensor_scalar(
                        out=cm[:], in0=s_reps[b][:],
                        scalar1=s_parts[b][:, c : c + 1], scalar2=0.0,
                        op0=mybir.AluOpType.is_gt, op1=mybir.AluOpType.add,
                        accum_out=rank_parts[b][:, c : c + 1],
                    )
            for b in range(B_LOC):
                masks[b] = sb.tile([128, NB], F16, name=f"mask{b}", tag=f"mask{b}")
                nc.vector.tensor_scalar(
                    out=masks[b][:], in0=rank_parts[b][:], scalar1=float(TK) - 0.5,
                    scalar2=None, op0=mybir.AluOpType.is_lt,
                )

            def matvec(b, w16, tag):
                """PSUM[128, NB] <- K w  (16 accumulating [128,128]x[128,1])."""
                pw = ps_w.tile([128, NB], F32, name=f"pw{b}", tag=tag)
                for io in range(NB):
                    for jo in range(NB):
                        nc.tensor.matmul(
                            pw[:, io : io + 1],
                            kts[b][:, jo * N + io * 128 : jo * N + (io + 1) * 128],
                            w16[:, jo : jo + 1],
                            start=(jo == 0),
                            stop=(jo == NB - 1),
                        )
                return pw

            # ---- damped rounds: W <- 0.5 W + reciprocal(K W) ----
            w16 = {}
            for b in range(B_LOC):
                w0 = wp.tile([128, NB], F16, name=f"w{b}", tag=f"w{b}")
                nc.vector.memset(w0[:], SQ2)
                w16[b] = w0
            for k in range(ROUNDS):
                for b in range(B_LOC):
                    pw = matvec(b, w16[b], f"pw{b}")
                    r = scr.tile([128, NB], F32, name=f"r{b}", tag=f"r{b}")
                    nc.vector.reciprocal(r[:], pw[:])
                    wn = wp.tile([128, NB], F16, name=f"w{b}", tag=f"w{b}")
                    nc.vector.scalar_tensor_tensor(
                        out=wn[:], in0=w16[b][:], scalar=0.5, in1=r[:],
                        op0=mybir.AluOpType.mult, op1=mybir.AluOpType.add,
                    )
                    w16[b] = wn

            # ---- output: u = W, v = 1/(K W), out = v * (K (mask*W)).
            # One fused 2-column matvec per batch computes K W and
            # K (mask*W) together (moving cols interleaved), then strided
            # APs pick the halves.  Results are PE-transposed into one
            # [4, 2*128] tile and leave through a single DMA. ----
            pso = ps_t.tile([4, B_LOC * 128], F32, name="pso", tag="pso")
            out_fs = {}
            for b in range(B_LOC):
                w2 = sb.tile([128, 2 * NB], F16, name=f"w2_{b}", tag=f"w2_{b}")
                nc.vector.tensor_copy(w2[:, 0 : 2 * NB : 2], w16[b][:])
                nc.vector.tensor_tensor(
                    out=w2[:, 1 : 2 * NB : 2], in0=masks[b][:], in1=w16[b][:],
                    op=mybir.AluOpType.mult,
                )
                pw2 = ps_w.tile([128, 2 * NB], F32, name=f"pw2_{b}", tag=f"pw{b}")
                for io in range(NB):
                    for jo in range(NB):
                        nc.tensor.matmul(
                            pw2[:, 2 * io : 2 * io + 2],
                            kts[b][:, jo * N + io * 128 : jo * N + (io + 1) * 128],
                            w2[:, 2 * jo : 2 * jo + 2],
                            start=(jo == 0),
                            stop=(jo == NB - 1),
                        )
                rc2 = sb.tile([128, 2 * NB], F32, name=f"rc2_{b}", tag=f"rc2_{b}")
                nc.vector.reciprocal(rc2[:], pw2[:])
                out_fs[b] = sb.tile([128, NB], F32, name=f"of{b}", tag=f"of{b}")
                nc.vector.tensor_tensor(
                    out=out_fs[b][:], in0=pw2[:, 1 : 2 * NB : 2],
                    in1=rc2[:, 0 : 2 * NB : 2], op=mybir.AluOpType.mult,
                )
                nc.tensor.transpose(
                    pso[:, b * 128 : (b + 1) * 128], out_fs[b][:], eye128[:]
                )
            o_sb = sb.tile([4, B_LOC * 128], F32, name="o_sb", tag="o_sb")
            nc.vector.tensor_copy(o_sb[:], pso[:])
            nc.sync.dma_start(out_d[:], o_sb[:])

    nc.compile()
    return nc


_NC_CACHE = []


def kernel(scores):
    scores = np.ascontiguousarray(np.asarray(scores, dtype=np.float32))
    assert scores.shape == (B_FULL, N)
    for b in range(B_FULL):
        # the comparison-count ranks assume distinct scores per batch
        assert np.unique(scores[b]).size == N, "tied scores unsupported"
    if not _NC_CACHE:
        _NC_CACHE.append(_build())
    nc = _NC_CACHE[0]

    eye128 = np.eye(128, dtype=np.float32)
    in_maps = []
    for c in range(N_CORES):
        sh = scores[c * B_LOC : (c + 1) * B_LOC]
        in_maps.append({"scores": sh, "eye128": eye128})
    res = run_bass_kernel_spmd(nc, in_maps, core_ids=list(range(N_CORES)))
    # device out[p, b*128+c] = out_full[b, c*128+p]
    outs = []
    for c in range(N_CORES):
        arr = res.results[c]["out"].reshape(4, B_LOC, 128)
        outs.append(arr.transpose(1, 0, 2).reshape(B_LOC, N))
    return np.concatenate(outs, axis=0).astype(np.float32)


# revision 13
# speedup vs baseline: 1.0428x; 1.0428x over previous
"""Differentiable top-k (Sinkhorn) Trainium2 kernel.

Math: the reference runs 100 log-domain Sinkhorn iterations on
log_P0[i,j] = -(s_i - sorted_j)^2/eps, then sums exp(log_P) over the
first K=50 columns.

This kernel exploits three structural facts:

1. Conjugating by the sort permutation, the whole iteration runs in the
   RAW index domain on the symmetric kernel matrix
   K[i,j] = exp(-(s_i - s_j)^2/eps); no on-chip sort is needed.  The
   only sorted-domain quantity required is the top-50 mask
   m[j] = [rank_j < 50], obtained by comparison counting.

2. The alternating column/row normalizations collapse to the scalar
   chain w <- 1/(K w); its linearization has spectrum {1} U [0, ~0.977],
   so the DAMPED iteration  w <- 0.5 w + 0.5/(K w)  contracts every
   error mode at <= ~0.5 per step instead of 0.977: three damped rounds
   land within rv ~ 4e-6 of the T=100 reference output (the T=100
   truncation transient itself is only ~6e-3 absmax from the fixed
   point, far inside the 2e-2 gate).  Storing W = sqrt(2) w makes the
   update exactly  W <- 0.5 W + reciprocal(K W)  (one vector reciprocal
   + one scalar_tensor_tensor); the sqrt(2) gauge cancels in the output
   because out = v * (K (m*u)) is invariant under u -> c u, v -> v/c.

3. Output: u = W_3, v = reciprocal(K W_3),
   out = v * (K (m*W_3)), all elementwise in raw order.

Latency engineering (the kernel is dependency-bound, not
throughput-bound): a dummy activation hoists the ~2.7us ACT table load
to t=0; the score row-replica s_rep is built by a PE broadcast matmul
(ones^T @ s_row) straight into PSUM (a 256KB HBM DMA takes ~5.6us, the
matmul 1.5us); score-layout transposes run on the PE; both batches'
outputs are PE-transposed into one [4, 256] tile and leave through a
single contiguous DMA; input DMAs are spread across the three DMA
queues (sync/gpsimd/scalar).
"""

import numpy as np

import concourse.bacc as bacc
import concourse.mybir as mybir
from concourse import bass, tile
from concourse.bass_utils import run_bass_kernel_spmd

F32 = mybir.dt.float32
F16 = mybir.dt.float16
BF16 = mybir.dt.bfloat16

B_FULL = 16
N = 512
NB = N // 128  # 4 row/col blocks
TK = 50
N_CORES = 8
B_LOC = B_FULL // N_CORES  # batches per core
ROUNDS = 3  # damped w-update rounds (then +1 matvec for v, +1 for out)
SQ2 = float(np.sqrt(2.0))
RT1000 = float(np.sqrt(1000.0))  # sqrt(1/eps)


def _build():
    nc = bacc.Bacc("TRN2", target_bir_lowering=False, debug=False)

    scores_d = nc.declare_dram_parameter("scores", [B_LOC, N], F32, isOutput=False)
    eye128_d = nc.declare_dram_parameter("eye128", [128, 128], F32, isOutput=False)
    # out_flat[p, b*128 + c] = out_full[b, c*128 + p]
    out_d = nc.declare_dram_parameter("out", [4, B_LOC * 128], F32, isOutput=True)

    with nc.allow_low_precision(reason="fp16 sinkhorn iterates"), \
         tile.TileContext(nc) as tc:
        with tc.tile_pool(name="sb", bufs=1) as sb, \
             tc.tile_pool(name="scr", bufs=2) as scr, \
             tc.tile_pool(name="wp", bufs=2) as wp, \
             tc.tile_pool(name="ps_w", bufs=1, space="PSUM") as ps_w, \
             tc.tile_pool(name="ps_r", bufs=1, space="PSUM") as ps_r, \
             tc.tile_pool(name="ps_t", bufs=1, space="PSUM") as ps_t:

            # ---- table-load hoist: dummy activation touching only zero_col
            zero_col = sb.tile([128, 1], F32, name="zero_col", tag="zero_col")
            nc.vector.memset(zero_col[:], 0.0)
            dummy = sb.tile([128, 1], F32, name="dummy", tag="dummy")
            nc.scalar.activation(
                dummy[:], zero_col[:], mybir.ActivationFunctionType.Square,
                bias=0.0, scale=1.0,
            )

            # ---- input DMAs.  Only the [4, 128] row layouts are loaded
            # (4 fat descriptors each; a [1, 512] row load would shatter
            # into 16x128B descriptors at ~170ns apiece). ----
            rows4 = {}
            for b in range(B_LOC):
                rows4[b] = sb.tile([4, 128], F32, name=f"rows4_{b}", tag=f"rows4_{b}")
            eye4 = sb.tile([4, 4], F32, name="eye4", tag="eye4")
            ones4 = sb.tile([4, 4], F32, name="ones4", tag="ones4")
            ones4w = sb.tile([4, 128], F32, name="ones4w", tag="ones4w")
            eye128 = sb.tile([128, 128], F32, name="eye128", tag="eye128")
            # eye4 built on-chip (gpsimd): no DMA on the scalar queue, and
            # the PE transposes are not gated on a late identity load
            nc.gpsimd.memset(ones4[:], 1.0)
            nc.gpsimd.affine_select(
                eye4[:], ones4[:], [[1, 4]], mybir.AluOpType.is_equal, 0.0,
                base=0, channel_multiplier=-1,
            )
            nc.vector.memset(ones4w[:], 1.0)
            # block-diagonal moving operands for the broadcast matmuls:
            # bd[k, k*128:(k+1)*128] = scores chunk k, written by a single
            # diagonal-scatter DMA (custom AP: partition step +1 row and
            # +128 elements), zeros elsewhere
            bds = {}
            for b in range(B_LOC):
                bds[b] = sb.tile([4, N], F32, name=f"bd{b}", tag=f"bd{b}")
                nc.vector.memset(bds[b][:], 0.0)
            diag0 = bass.AP(bds[0][:].tensor, 0, [[N + 128, 4], [1, 128]])
            diag1 = bass.AP(bds[1][:].tensor, 0, [[N + 128, 4], [1, 128]])
            nc.sync.dma_start(diag0, scores_d[0].rearrange("(p c) -> p c", p=4))
            nc.gpsimd.dma_start(diag1, scores_d[1].rearrange("(p c) -> p c", p=4))
            nc.sync.dma_start(rows4[0][:], scores_d[0].rearrange("(p c) -> p c", p=4))
            nc.gpsimd.dma_start(rows4[1][:], scores_d[1].rearrange("(p c) -> p c", p=4))
            nc.gpsimd.dma_start(eye128[:], eye128_d[:])

            # ---- s_rep[b] = ones^T @ s_row (PSUM [128, N] broadcast), and
            #      s_part[p, c] = s[c*128+p] via PE transpose.  Separate
            #      PSUM tiles per batch: tile-granular dependency tracking
            #      must not make batch 0's consumers wait on batch 1. ----
            s_reps, s_parts, nsqs = {}, {}, {}
            for b in range(B_LOC):
                # ones^T @ bd sums the single nonzero per column,
                # broadcasting s across all 128 partitions
                s_reps[b] = ps_r.tile([128, N], F32, name=f"srep{b}", tag=f"srep{b}")
                nc.tensor.matmul(
                    s_reps[b][:], ones4w[:], bds[b][:], start=True, stop=True
                )
                pt = ps_t.tile([128, 4], F32, name=f"pst{b}", tag=f"pst{b}")
                nc.tensor.transpose(pt[:], rows4[b][:], eye4[:])
                nsqs[b] = sb.tile([128, NB], F32, name=f"nsq{b}", tag=f"nsq{b}")
                nc.vector.tensor_scalar(
                    out=nsqs[b][:], in0=pt[:], scalar1=-RT1000,
                    scalar2=None, op0=mybir.AluOpType.mult,
                )
                s_parts[b] = sb.tile([128, NB], F32, name=f"s_part{b}", tag=f"s_part{b}")
                nc.vector.tensor_copy(s_parts[b][:], pt[:])

            # ---- K tiles on scalar: 4 biased Squares + one fused Exp ----
            # kt[b][q, jo*N + i] = exp(-1000 (s_{jo*128+q} - s_i)^2)
            kts = {}
            for b in range(B_LOC):
                sq_all = scr.tile([128, NB * N], F32, name="sq_all", tag="sq_all", bufs=1)
                for jo in range(NB):
                    nc.scalar.activation(
                        sq_all[:, jo * N : (jo + 1) * N], s_reps[b][:],
                        mybir.ActivationFunctionType.Square,
                        bias=nsqs[b][:, jo : jo + 1], scale=RT1000,
                    )
                kts[b] = sb.tile([128, NB * N], F16, name=f"kt{b}", tag=f"kt{b}")
                nc.scalar.activation(
                    kts[b][:], sq_all[:], mybir.ActivationFunctionType.Exp,
                    bias=zero_col[:], scale=-1.0,
                )

            # ---- ranks on vector: rank[j] = #{i: s_i > s_j}, then mask ----
            rank_parts, masks = {}, {}
            for b in range(B_LOC):
                rank_parts[b] = sb.tile([128, NB], F32, name=f"rank{b}", tag=f"rank{b}")
            for c in range(NB):
                for b in range(B_LOC):
                    cm = scr.tile(
                        [128, N], BF16, name=f"cmp{b}_{c}", tag=f"cmp{b}_{c % 2}"
                    )
                    nc.vector.tensor_scalar(
                        out=cm[:], in0=s_reps[b][:],
                        scalar1=s_parts[b][:, c : c + 1], scalar2=0.0,
                        op0=mybir.AluOpType.is_gt, op1=mybir.AluOpType.add,
                        accum_out=rank_parts[b][:, c : c + 1],
                    )
            for b in range(B_LOC):
                masks[b] = sb.tile([128, NB], F16, name=f"mask{b}", tag=f"mask{b}")
                nc.vector.tensor_scalar(
                    out=masks[b][:], in0=rank_parts[b][:], scalar1=float(TK) - 0.5,
                    scalar2=None, op0=mybir.AluOpType.is_lt,
                )

            def matvec(b, w16, tag):
                """PSUM[128, NB] <- K w  (16 accumulating [128,128]x[128,1])."""
                pw = ps_w.tile([128, NB], F32, name=f"pw{b}", tag=tag)
                for io in range(NB):
                    for jo in range(NB):
                        nc.tensor.matmul(
                            pw[:, io : io + 1],
                            kts[b][:, jo * N + io * 128 : jo * N + (io + 1) * 128],
                            w16[:, jo : jo + 1],
                            start=(jo == 0),
                            stop=(jo == NB - 1),
                        )
                return pw

            # ---- damped rounds: W <- 0.5 W + reciprocal(K W) ----
            w16 = {}
            for b in range(B_LOC):
                w0 = wp.tile([128, NB], F16, name=f"w{b}", tag=f"w{b}")
                nc.vector.memset(w0[:], SQ2)
                w16[b] = w0
            for k in range(ROUNDS):
                for b in range(B_LOC):
                    pw = matvec(b, w16[b], f"pw{b}")
                    r = scr.tile([128, NB], F32, name=f"r{b}", tag=f"r{b}")
                    nc.vector.reciprocal(r[:], pw[:])
                    wn = wp.tile([128, NB], F16, name=f"w{b}", tag=f"w{b}")
                    nc.vector.scalar_tensor_tensor(
                        out=wn[:], in0=w16[b][:], scalar=0.5, in1=r[:],
                        op0=mybir.AluOpType.mult, op1=mybir.AluOpType.add,
                    )
                    w16[b] = wn

            # ---- output: u = W, v = 1/(K W), out = v * (K (mask*W)).
            # One fused 2-column matvec per batch computes K W and
            # K (mask*W) together (moving cols interleaved), then strided
            # APs pick the halves.  Results are PE-transposed into one
            # [4, 2*128] tile and leave through a single DMA. ----
            pso = ps_t.tile([4, B_LOC * 128], F32, name="pso", tag="pso")
            out_fs = {}
            for b in range(B_LOC):
                w2 = sb.tile([128, 2 * NB], F16, name=f"w2_{b}", tag=f"w2_{b}")
                nc.vector.tensor_copy(w2[:, 0 : 2 * NB : 2], w16[b][:])
                nc.vector.tensor_tensor(
                    out=w2[:, 1 : 2 * NB : 2], in0=masks[b][:], in1=w16[b][:],
                    op=mybir.AluOpType.mult,
                )
                pw2 = ps_w.tile([128, 2 * NB], F32, name=f"pw2_{b}", tag=f"pw{b}")
                for io in range(NB):
                    for jo in range(NB):
                        nc.tensor.matmul(
                            pw2[:, 2 * io : 2 * io + 2],
                            kts[b][:, jo * N + io * 128 : jo * N + (io + 1) * 128],
                            w2[:, 2 * jo : 2 * jo + 2],
                            start=(jo == 0),
                            stop=(jo == NB - 1),
                        )
                rc2 = sb.tile([128, 2 * NB], F32, name=f"rc2_{b}", tag=f"rc2_{b}")
                nc.vector.reciprocal(rc2[:], pw2[:])
                out_fs[b] = sb.tile([128, NB], F32, name=f"of{b}", tag=f"of{b}")
                nc.vector.tensor_tensor(
                    out=out_fs[b][:], in0=pw2[:, 1 : 2 * NB : 2],
                    in1=rc2[:, 0 : 2 * NB : 2], op=mybir.AluOpType.mult,
                )
                nc.tensor.transpose(
                    pso[:, b * 128 : (b + 1) * 128], out_fs[b][:], eye128[:]
                )
            o_sb = sb.tile([4, B_LOC * 128], F32, name="o_sb", tag="o_sb")
            nc.vector.tensor_copy(o_sb[:], pso[:])
            nc.sync.dma_start(out_d[:], o_sb[:])

    nc.compile()
    return nc


_NC_CACHE = []


def kernel(scores):
    scores = np.ascontiguousarray(np.asarray(scores, dtype=np.float32))
    assert scores.shape == (B_FULL, N)
    for b in range(B_FULL):
        # the comparison-count ranks assume distinct scores per batch
        assert np.unique(scores[b]).size == N, "tied scores unsupported"
    if not _NC_CACHE:
        _NC_CACHE.append(_build())
    nc = _NC_CACHE[0]

    eye128 = np.eye(128, dtype=np.float32)
    in_maps = []
    for c in range(N_CORES):
        sh = scores[c * B_LOC : (c + 1) * B_LOC]
        in_maps.append({"scores": sh, "eye128": eye128})
    res = run_bass_kernel_spmd(nc, in_maps, core_ids=list(range(N_CORES)))
    # device out[p, b*128+c] = out_full[b, c*128+p]
    outs = []
    for c in range(N_CORES):
        arr = res.results[c]["out"].reshape(4, B_LOC, 128)
        outs.append(arr.transpose(1, 0, 2).reshape(B_LOC, N))
    return np.concatenate(outs, axis=0).astype(np.float32)
